# revision 1
# baseline (speedup 1.0000x reference)
"""Classical self-attention (head-summed scores) on 8 trn2 NeuronCores.

Math (per batch b):
    Q = x Wq; K = x Wk; V = x Wv          (W_qkv split columns 3x1024)
    S = Q K^T / 8   (full-E contraction: heads+dims summed)
    P = softmax(S, axis=-1)
    out = (P V) W_out + b_out

Sharding: 8 cores = (4 batches) x (2 query-halves). Each core gets its
batch's x rotated so its 1024 query rows come first; keys are the full
2048 rows (key order is irrelevant to the result). K/V projections are
duplicated between the 2 cores of a batch; no collectives needed.

Per-core kernel layout strategy:
  - S^T layout (keys on partitions) so the softmax reduction over keys
    becomes a ones-matmul and P^T feeds the O^T matmuls directly.
  - Softmax skips the max-subtraction (scores ~ N(0,4): exp stays well
    inside fp32 range); normalization by 1/rowsum is deferred to the
    final output projection where query rows sit on partitions.
  - All big matmuls in fp32r (tf32 datapath, full rate at free dim>=256).
  - K^T and V staged through internal DRAM to stay under SBUF; Q^T stays
    SBUF-resident so the scores phase overlaps the projection phase.
"""

import sys

sys.path.insert(0, "/opt/trn_rl_repo")

import numpy as np

import concourse.bass as bass
import concourse.mybir as mybir
import concourse.tile as tile
from concourse import bacc
from concourse.masks import make_identity

B, N, E = 4, 2048, 1024
NQ = N // 2          # query rows per core
P = 128              # partitions
FT = E // P          # 8 feature tiles (contraction for projections)
ET = E // P          # 8 embed tiles
MT = N // P          # 16 key tiles
QT = NQ // P         # 8 query tiles
MB = 4               # key tiles per projection block
NBLK = MT // MB      # 4 blocks
F32 = mybir.dt.float32
F32R = mybir.dt.float32r


def build_program():
    nc = bacc.Bacc("TRN2", target_bir_lowering=False, debug=False)
    x = nc.dram_tensor("x", [N, E], F32, kind="ExternalInput").ap()
    wqkv = nc.dram_tensor("wqkv", [E, 3 * E], F32, kind="ExternalInput").ap()
    wout = nc.dram_tensor("wout", [E, E], F32, kind="ExternalInput").ap()
    bout = nc.dram_tensor("bout", [E], F32, kind="ExternalInput").ap()
    y = nc.dram_tensor("y", [NQ, E], F32, kind="ExternalOutput").ap()

    with tile.TileContext(nc) as tc:
        _body(nc, tc, x, wqkv, wout, bout, y)
    nc.compile()
    return nc


def _body(nc, tc, x, wqkv, wout, bout, y):
    with tc.tile_pool(name="dram", bufs=1, space="DRAM") as dramp:
        kT_d = dramp.tile([E, N], F32R, name="kT_d", tag="kT_d")
        v_d = dramp.tile([N, E], F32R, name="v_d", tag="v_d")

        qTp = tc.alloc_tile_pool(name="qTp", bufs=1)
        qT = [qTp.tile([P, NQ], F32R, name=f"qT{e}", tag=f"qT{e}")
              for e in range(ET)]

        _phase_project(nc, tc, x, wqkv, kT_d, v_d, qT)

        # W_out / b_out tiles; DMAs issued at phase_scores start.
        wop = tc.alloc_tile_pool(name="wo", bufs=1)
        wo = [wop.tile([P, E], F32R, name=f"wo{e}", tag=f"wo{e}")
              for e in range(ET)]
        bo_b = wop.tile([P, E], F32, name="bo_b", tag="bo_b")
        bout_bcast = bass.AP(tensor=bout.tensor, offset=0,
                             ap=[[0, P], [1, E]])
        for e in range(ET):
            nc.gpsimd.dma_start(out=wo[e], in_=wout[e * P:(e + 1) * P, :])
        nc.sync.dma_start(out=bo_b, in_=bout_bcast)

        p_tiles, recip, pres, recp = _phase_scores(nc, tc, kT_d, qT, [])
        oT, oTp = _phase_pv(nc, tc, p_tiles, v_d, pres)
        _phase_out(nc, tc, oT, recip, wo, bo_b, y)
        wop.release()
        qTp.release()
        oTp.release()
        recp.release()


def _phase_project(nc, tc, x, wqkv, kT_d, v_d, qT):
    """x -> x^T (PE transpose), then K^T (to DRAM), Q^T (SBUF), V (DRAM)."""
    with tc.tile_pool(name="wconst", bufs=1) as wcp, \
         tc.tile_pool(name="xin", bufs=3) as xp, \
         tc.tile_pool(name="xT", bufs=2) as xTp, \
         tc.tile_pool(name="ktmp", bufs=2) as ktp, \
         tc.tile_pool(name="vtmp", bufs=2) as vtp, \
         tc.tile_pool(name="tpps", bufs=2, space="PSUM") as tpp, \
         tc.tile_pool(name="pjps", bufs=4, space="PSUM") as pjp:

        ident = wcp.tile([P, P], F32, name="ident", tag="ident")
        make_identity(nc, ident)

        # Wk first: the first projection matmuls need it soonest.
        wk, wq, wv = [], [], []
        for lst, nm, c0 in ((wk, "wk", E), (wq, "wq", 0), (wv, "wv", 2 * E)):
            for f in range(FT):
                t = wcp.tile([P, E], F32R, name=f"{nm}{f}", tag=f"{nm}{f}")
                nc.gpsimd.dma_start(
                    out=t, in_=wqkv[f * P:(f + 1) * P, c0:c0 + E])
                lst.append(t)

        for blk in range(NBLK):
            xT = xTp.tile([P, FT, MB * P], F32R, name="xT", tag="xT")
            for mt in range(MB):
                m = blk * MB + mt
                xt = xp.tile([P, E], F32, name="xt", tag="xt")
                nc.sync.dma_start(out=xt, in_=x[m * P:(m + 1) * P, :])
                for f in range(FT):
                    tp = tpp.tile([P, P], F32, name="tp", tag="tp")
                    nc.tensor.transpose(tp, xt[:, f * P:(f + 1) * P], ident)
                    nc.vector.tensor_copy(xT[:, f, mt * P:(mt + 1) * P], tp)

            # K^T block (all e rows, this block's key columns)
            for e in range(ET):
                ps = pjp.tile([P, MB * P], F32, name="pjk", tag="pj")
                for f in range(FT):
                    nc.tensor.matmul(ps, wk[f][:, e * P:(e + 1) * P],
                                     xT[:, f, :],
                                     start=(f == 0), stop=(f == FT - 1))
                kt_sb = ktp.tile([P, MB * P], F32R, name="kt_sb", tag="kt_sb")
                nc.vector.tensor_copy(kt_sb, ps)
                nc.sync.dma_start(
                    out=kT_d[e * P:(e + 1) * P, blk * MB * P:(blk + 1) * MB * P],
                    in_=kt_sb)

            # Q^T block straight into resident SBUF tiles
            if blk * MB * P < NQ:
                for e in range(ET):
                    ps = pjp.tile([P, MB * P], F32, name="pjq", tag="pj")
                    for f in range(FT):
                        nc.tensor.matmul(ps, wq[f][:, e * P:(e + 1) * P],
                                         xT[:, f, :],
                                         start=(f == 0), stop=(f == FT - 1))
                    nc.vector.tensor_copy(
                        qT[e][:, blk * MB * P:(blk + 1) * MB * P], ps)

            # V block (natural layout rows) to DRAM
            for mt in range(MB):
                m = blk * MB + mt
                vt = vtp.tile([P, E], F32R, name="vt", tag="vt")
                for h in range(2):
                    ps = pjp.tile([P, E // 2], F32, name="pjv", tag="pj")
                    for f in range(FT):
                        nc.tensor.matmul(
                            ps, xT[:, f, mt * P:(mt + 1) * P],
                            wv[f][:, h * (E // 2):(h + 1) * (E // 2)],
                            start=(f == 0), stop=(f == FT - 1))
                    nc.vector.tensor_copy(
                        vt[:, h * (E // 2):(h + 1) * (E // 2)], ps)
                nc.sync.dma_start(out=v_d[m * P:(m + 1) * P, :], in_=vt)


def _phase_scores(nc, tc, kT_d, qT, wo_loads):
    """S^T = K^T.T Q^T per key tile; P^T = exp(S^T/8); rowsums via ones-matmul."""
    kT_r = kT_d.rearrange("(e p) m -> p e m", p=P)
    recp = tc.alloc_tile_pool(name="recp", bufs=1, side="right")
    pres = tc.alloc_tile_pool(name="pres", bufs=1)
    with tc.tile_pool(name="kts", bufs=3) as ktsp, \
         tc.tile_pool(name="small", bufs=1) as smp, \
         tc.tile_pool(name="sps", bufs=3, space="PSUM") as sp, \
         tc.tile_pool(name="sumps", bufs=2, space="PSUM") as sumsp:

        ones = smp.tile([P, 1], F32, name="ones", tag="ones")
        nc.vector.memset(ones, 1.0)
        sums_acc = smp.tile([P, QT], F32, name="sums_acc", tag="sums_acc")

        p_tiles = []
        for m in range(MT):
            kt = ktsp.tile([P, ET, P], F32R, name="kt", tag="kt")
            nc.sync.dma_start(out=kt, in_=kT_r[:, :, m * P:(m + 1) * P])
            s = sp.tile([P, NQ], F32, name="s", tag="s")
            for e in range(ET):
                for h in range(2):
                    nc.tensor.matmul(
                        s[:, h * (NQ // 2):(h + 1) * (NQ // 2)],
                        kt[:, e, :],
                        qT[e][:, h * (NQ // 2):(h + 1) * (NQ // 2)],
                        start=(e == 0), stop=(e == ET - 1))
            p = pres.tile([P, NQ], F32R, name=f"p{m}", tag=f"p{m}")
            nc.scalar.activation(p, s, mybir.ActivationFunctionType.Exp,
                                 scale=0.125)
            p_tiles.append(p)
            # Row-sum the PREVIOUS tile's exp: its activation ran while
            # this tile's S matmuls were on PE, so PE never waits on ACT.
            if m > 0:
                _row_sums(nc, p_tiles[m - 1], sumsp, smp, ones, sums_acc,
                          first=(m == 1))
        _row_sums(nc, p_tiles[MT - 1], sumsp, smp, ones, sums_acc,
                  first=False)

        recip = recp.tile([P, QT], F32, name="recip", tag="recip")
        nc.vector.reciprocal(recip, sums_acc)

    return p_tiles, recip, pres, recp


def _row_sums(nc, p, sumsp, smp, ones, sums_acc, first):
    sums_m = sumsp.tile([P, QT], F32, name="sums_m", tag="sums_m")
    for q in range(QT):
        nc.tensor.matmul(sums_m[:, q:q + 1],
                         p[:, q * P:(q + 1) * P].bitcast(F32), ones,
                         start=True, stop=True)
    if first:
        nc.vector.tensor_copy(sums_acc, sums_m)
    else:
        nc.vector.tensor_tensor(out=sums_acc, in0=sums_acc,
                                in1=sums_m, op=mybir.AluOpType.add)


def _phase_pv(nc, tc, p_tiles, v_d, pres):
    """O^T[e, nq] = sum_m V[m,e]^T P^T[m,nq], accumulated in PSUM.

    e-tiles are processed in 2 groups of 4 so each group's O^T rows fit
    in PSUM ([128, NQ] x 4 = 8 banks) and V streams from DRAM only once
    per group (half its columns each time).
    """
    oTp = tc.alloc_tile_pool(name="oTp", bufs=1, side="right")
    oT = [oTp.tile([P, NQ], F32R, name=f"oT{e}", tag=f"oT{e}")
          for e in range(ET)]
    EG = ET // 2
    H = NQ // 2
    with tc.tile_pool(name="vstream", bufs=4) as vsp, \
         tc.tile_pool(name="ops", bufs=1, space="PSUM") as opp:
        for g in range(2):
            o_ps = [opp.tile([P, NQ], F32, name=f"o{j}", tag=f"o{j}")
                    for j in range(EG)]
            for m in range(MT):
                vt = vsp.tile([P, EG * P], F32R, name="vs", tag="vs")
                nc.sync.dma_start(
                    out=vt,
                    in_=v_d[m * P:(m + 1) * P, g * EG * P:(g + 1) * EG * P])
                for j in range(EG):
                    for h in range(2):
                        nc.tensor.matmul(
                            o_ps[j][:, h * H:(h + 1) * H],
                            vt[:, j * P:(j + 1) * P],
                            p_tiles[m][:, h * H:(h + 1) * H],
                            start=(m == 0), stop=(m == MT - 1))
            for j in range(EG):
                nc.vector.tensor_copy(oT[g * EG + j], o_ps[j])
    pres.release()
    return oT, oTp


def _phase_out(nc, tc, oT, recip, wo, bo_b, y):
    """y rows = (O_u W_out) * recip + b_out."""
    with tc.tile_pool(name="ysb", bufs=3) as ysp, \
         tc.tile_pool(name="yps", bufs=2, space="PSUM") as ypp:

        H = E // 2
        for nqt in range(QT):
            yps = ypp.tile([P, E], F32, name="yps", tag="yps")
            for e in range(ET):
                for h in range(2):
                    nc.tensor.matmul(
                        yps[:, h * H:(h + 1) * H],
                        oT[e][:, nqt * P:(nqt + 1) * P],
                        wo[e][:, h * H:(h + 1) * H],
                        start=(e == 0), stop=(e == ET - 1))
            ysb = ysp.tile([P, E], F32, name="ysb", tag="ysb")
            nc.vector.tensor_scalar_mul(ysb, yps, recip[:, nqt:nqt + 1])
            nc.vector.tensor_tensor(out=ysb, in0=ysb, in1=bo_b,
                                    op=mybir.AluOpType.add)
            nc.sync.dma_start(out=y[nqt * P:(nqt + 1) * P, :], in_=ysb)


_NC_CACHE = None


def _get_program():
    global _NC_CACHE
    if _NC_CACHE is None:
        _NC_CACHE = build_program()
    return _NC_CACHE


def kernel(x, W_qkv, W_out, b_out):
    from concourse.bass_utils import run_bass_kernel_spmd

    x = np.asarray(x, dtype=np.float32)
    W_qkv = np.asarray(W_qkv, dtype=np.float32)
    W_out = np.asarray(W_out, dtype=np.float32)
    b_out = np.asarray(b_out, dtype=np.float32)

    nc = _get_program()
    in_maps = []
    for c in range(8):
        b, half = divmod(c, 2)
        xb = x[b]
        xrot = np.ascontiguousarray(
            np.concatenate([xb[half * NQ:], xb[:half * NQ]], axis=0))
        in_maps.append({"x": xrot, "wqkv": W_qkv, "wout": W_out,
                       "bout": b_out})
    res = run_bass_kernel_spmd(nc, in_maps, list(range(8)))
    out = np.empty((B, N, E), dtype=np.float32)
    for c in range(8):
        b, half = divmod(c, 2)
        out[b, half * NQ:(half + 1) * NQ] = res.results[c]["y"]
    return out



# revision 5
# speedup vs baseline: 1.2171x; 1.2171x over previous
"""Classical self-attention (head-summed scores) on 8 trn2 NeuronCores.

Math (per batch b):
    Q = x Wq; K = x Wk; V = x Wv          (W_qkv split columns 3x1024)
    S = Q K^T / 8   (full-E contraction: heads+dims summed)
    P = softmax(S, axis=-1)
    out = (P V) W_out + b_out

Sharding: 8 cores = (4 batches) x (2 query-halves). Each core gets its
batch's x rotated so its 1024 query rows come first; keys are the full
2048 rows (key order is irrelevant to the result). K/V projections are
duplicated between the 2 cores of a batch; no collectives needed.

v2 design (vs v1):
  - Inputs host-converted to bf16: halves DMA and SBUF, same PE rate
    (1.0 cycles/row) as fp32r in the TRN2 cost model.
  - x^T produced by DMA-transpose (XBAR) straight from DRAM: no PE
    transposes, no DVE copies, no x staging in SBUF.
  - K^T, V, Q^T, P all SBUF-resident in bf16 -- no DRAM staging
    roundtrips at all.
  - Projection warmup: first 4 K^T chunks run f-outer so PE streams
    while x^T/Wk tiles are still arriving from DRAM.
  - Softmax skips max-subtraction (scores ~ N(0,16) after 1/8 scale:
    exp stays in fp32/bf16 range); 1/rowsum deferred to the output
    projection.
  - PSUM->SBUF copies split across DVE (K^T/Q^T/O^T) and ACT (V) so
    neither engine gates PSUM recycling.
"""

import sys

sys.path.insert(0, "/opt/trn_rl_repo")

import numpy as np

import concourse.bass as bass
import concourse.mybir as mybir
import concourse.tile as tile
from concourse import bacc

B, N, E = 4, 2048, 1024
NQ = N // 2          # query rows per core
P = 128              # partitions
FT = E // P          # 8 feature (contraction) tiles
ET = E // P          # 8 embed tiles
MT = N // P          # 16 key tiles
QT = NQ // P         # 8 query tiles
HKEY = N // 2        # 1024 keys per half (for x^T transpose granularity)
F32 = mybir.dt.float32
BF16 = mybir.dt.bfloat16
EXP = mybir.ActivationFunctionType.Exp


def build_program():
    nc = bacc.Bacc("TRN2", target_bir_lowering=False, debug=False)
    x = nc.dram_tensor("x", [N, E], BF16, kind="ExternalInput").ap()
    wqkv = nc.dram_tensor("wqkv", [E, 3 * E], BF16, kind="ExternalInput").ap()
    wout = nc.dram_tensor("wout", [E, E], BF16, kind="ExternalInput").ap()
    bout = nc.dram_tensor("bout", [E], F32, kind="ExternalInput").ap()
    y = nc.dram_tensor("y", [NQ, E], F32, kind="ExternalOutput").ap()

    with tile.TileContext(nc) as tc:
        _body(nc, tc, x, wqkv, wout, bout, y)
    nc.compile()
    return nc


def _body(nc, tc, x, wqkv, wout, bout, y):
    # ---- persistent SBUF residents (right side) --------------------------
    wop = tc.alloc_tile_pool(name="wo", bufs=1, side="right")
    wo = [wop.tile([P, E], BF16, name=f"wo{e}", tag=f"wo{e}") for e in range(ET)]
    bo_b = wop.tile([P, E], F32, name="bo_b", tag="bo_b")

    kqp = tc.alloc_tile_pool(name="kq", bufs=1, side="right")
    kT = [kqp.tile([P, N], BF16, name=f"kT{e}", tag=f"kT{e}") for e in range(ET)]
    qT = [kqp.tile([P, NQ], BF16, name=f"qT{e}", tag=f"qT{e}") for e in range(ET)]

    vp = tc.alloc_tile_pool(name="vp", bufs=1, side="right")
    v = [vp.tile([P, E], BF16, name=f"v{m}", tag=f"v{m}") for m in range(MT)]

    smp = tc.alloc_tile_pool(name="small", bufs=1, side="right")
    ones = smp.tile([P, 1], BF16, name="ones", tag="ones")
    sums_acc = smp.tile([P, QT], F32, name="sums_acc", tag="sums_acc")
    recip = smp.tile([P, QT], F32, name="recip", tag="recip")

    # ---- phase 1: load + project ----------------------------------------
    # Left side: wqkv under xT (LIFO release order: xT then wqkv).
    wp = tc.alloc_tile_pool(name="wqkv", bufs=1)
    wk = [wp.tile([P, E], BF16, name=f"wk{f}", tag=f"wk{f}") for f in range(FT)]
    wq = [wp.tile([P, E], BF16, name=f"wq{f}", tag=f"wq{f}") for f in range(FT)]
    wv = [wp.tile([P, E], BF16, name=f"wv{f}", tag=f"wv{f}") for f in range(FT)]

    xTp = tc.alloc_tile_pool(name="xT", bufs=1)
    # xT[f][h]: [128, 1024] = transpose of x[h*1024:(h+1)*1024, f*128:(f+1)*128]
    xT = [[xTp.tile([P, HKEY], BF16, name=f"xT{f}_{h}", tag=f"xT{f}_{h}")
           for h in range(2)] for f in range(FT)]

    # DMA issue order (all on SP queue; transfers serialize on the DMA
    # engines in this order): xT half0 + Wk interleaved (the warmup K
    # chunks consume them f-by-f), then xT half1, Wq, Wv, Wout, b_out.
    for f in range(FT):
        nc.sync.dma_start_transpose(
            out=xT[f][0], in_=x[0:HKEY, f * P:(f + 1) * P])
        nc.sync.dma_start(out=wk[f], in_=wqkv[f * P:(f + 1) * P, E:2 * E])
    for f in range(FT):
        nc.sync.dma_start_transpose(
            out=xT[f][1], in_=x[HKEY:N, f * P:(f + 1) * P])
    for f in range(FT):
        nc.sync.dma_start(out=wq[f], in_=wqkv[f * P:(f + 1) * P, 0:E])
    for f in range(FT):
        nc.sync.dma_start(out=wv[f], in_=wqkv[f * P:(f + 1) * P, 2 * E:3 * E])
    for e in range(ET):
        nc.sync.dma_start(out=wo[e], in_=wout[e * P:(e + 1) * P, :])
    bout_bcast = bass.AP(tensor=bout.tensor, offset=0, ap=[[0, P], [1, E]])
    nc.sync.dma_start(out=bo_b, in_=bout_bcast)
    nc.vector.memset(ones, 1.0)

    with tc.tile_pool(name="pjps", bufs=4, space="PSUM") as pjp:
        # Warmup: first 4 K chunks (e=0..3, keys half 0) f-outer, so each
        # matmul only needs xT[f][0] + wk[f] -- PE starts ~2us into the
        # DMA stream instead of waiting for all 8 f-tiles.
        warm = [pjp.tile([P, HKEY], F32, name=f"pw{e}", tag="pj")
                for e in range(4)]
        for f in range(FT):
            for e in range(4):
                nc.tensor.matmul(warm[e], wk[f][:, e * P:(e + 1) * P],
                                 xT[f][0], start=(f == 0), stop=(f == FT - 1))
        for e in range(4):
            nc.vector.tensor_copy(kT[e][:, 0:HKEY], warm[e])

        # Remaining K chunks (f-inner), then Q, then V.
        for (e, h) in [(e, h) for h in range(2) for e in range(ET)
                       if not (h == 0 and e < 4)]:
            ps = pjp.tile([P, HKEY], F32, name="pjk", tag="pj")
            for f in range(FT):
                nc.tensor.matmul(ps, wk[f][:, e * P:(e + 1) * P], xT[f][h],
                                 start=(f == 0), stop=(f == FT - 1))
            nc.vector.tensor_copy(kT[e][:, h * HKEY:(h + 1) * HKEY], ps)

        for e in range(ET):
            ps = pjp.tile([P, NQ], F32, name="pjq", tag="pj")
            for f in range(FT):
                nc.tensor.matmul(ps, wq[f][:, e * P:(e + 1) * P], xT[f][0],
                                 start=(f == 0), stop=(f == FT - 1))
            nc.vector.tensor_copy(qT[e], ps)

        for m in range(MT):
            h, mm = divmod(m, ET)
            ps = pjp.tile([P, E], F32, name="pjv", tag="pj")
            for f in range(FT):
                nc.tensor.matmul(ps, xT[f][h][:, mm * P:(mm + 1) * P], wv[f],
                                 start=(f == 0), stop=(f == FT - 1))
            nc.scalar.copy(out=v[m], in_=ps)

    xTp.release()
    wp.release()

    # ---- phase 2: scores + exp + row-sums (left side now free) ----------
    pp = tc.alloc_tile_pool(name="pp", bufs=1)
    p_tiles = [pp.tile([P, NQ], BF16, name=f"p{m}", tag=f"p{m}")
               for m in range(MT)]

    with tc.tile_pool(name="sps", bufs=3, space="PSUM") as sp, \
         tc.tile_pool(name="sumps", bufs=2, space="PSUM") as sumsp:
        for m in range(MT):
            s = sp.tile([P, NQ], F32, name="s", tag="s")
            for e in range(ET):
                nc.tensor.matmul(s, kT[e][:, m * P:(m + 1) * P], qT[e],
                                 start=(e == 0), stop=(e == ET - 1))
            nc.scalar.activation(p_tiles[m], s, EXP, scale=0.125)
            # Row-sum the PREVIOUS tile's exp while this tile's S matmuls
            # occupy PE, so PE never waits on ACT.
            if m > 0:
                _row_sums(nc, p_tiles[m - 1], sumsp, ones, sums_acc,
                          first=(m == 1))
        _row_sums(nc, p_tiles[MT - 1], sumsp, ones, sums_acc, first=False)
        nc.vector.reciprocal(recip, sums_acc)

    # ---- phase 3: O^T = sum_m V[m]^T P^T[m] ------------------------------
    oTp = tc.alloc_tile_pool(name="oTp", bufs=1)
    oT = [oTp.tile([P, NQ], BF16, name=f"oT{e}", tag=f"oT{e}")
          for e in range(ET)]
    EG = ET // 2
    with tc.tile_pool(name="ops", bufs=1, space="PSUM") as opp:
        for g in range(2):
            o_ps = [opp.tile([P, NQ], F32, name=f"o{j}", tag=f"o{j}")
                    for j in range(EG)]
            for m in range(MT):
                for j in range(EG):
                    e = g * EG + j
                    nc.tensor.matmul(o_ps[j], v[m][:, e * P:(e + 1) * P],
                                     p_tiles[m],
                                     start=(m == 0), stop=(m == MT - 1))
            # Alternate copy engines so group 1 gets PSUM banks back fast.
            for j in range(EG):
                e = g * EG + j
                if j % 2 == 0:
                    nc.vector.tensor_copy(oT[e], o_ps[j])
                else:
                    nc.scalar.copy(out=oT[e], in_=o_ps[j])

    # ---- phase 4: y = (O^T.T W_out) * recip + b_out ----------------------
    with tc.tile_pool(name="ysb", bufs=3) as ysp, \
         tc.tile_pool(name="yps", bufs=2, space="PSUM") as ypp:
        for nqt in range(QT):
            yps = ypp.tile([P, E], F32, name="yps", tag="yps")
            for e in range(ET):
                nc.tensor.matmul(yps, oT[e][:, nqt * P:(nqt + 1) * P], wo[e],
                                 start=(e == 0), stop=(e == ET - 1))
            ysb = ysp.tile([P, E], F32, name="ysb", tag="ysb")
            nc.vector.tensor_scalar_mul(ysb, yps, recip[:, nqt:nqt + 1])
            nc.gpsimd.tensor_add(out=ysb, in0=ysb, in1=bo_b)
            nc.sync.dma_start(out=y[nqt * P:(nqt + 1) * P, :], in_=ysb)

    oTp.release()
    pp.release()
    smp.release()
    vp.release()
    kqp.release()
    wop.release()


def _row_sums(nc, p, sumsp, ones, sums_acc, first):
    sums_m = sumsp.tile([P, QT], F32, name="sums_m", tag="sums_m")
    for q in range(QT):
        nc.tensor.matmul(sums_m[:, q:q + 1], p[:, q * P:(q + 1) * P], ones,
                         start=True, stop=True)
    if first:
        nc.vector.tensor_copy(sums_acc, sums_m)
    else:
        nc.vector.tensor_tensor(out=sums_acc, in0=sums_acc,
                                in1=sums_m, op=mybir.AluOpType.add)


_NC_CACHE = None


def _get_program():
    global _NC_CACHE
    if _NC_CACHE is None:
        _NC_CACHE = build_program()
    return _NC_CACHE


def kernel(x, W_qkv, W_out, b_out):
    import ml_dtypes
    from concourse.bass_utils import run_bass_kernel_spmd

    bf16 = ml_dtypes.bfloat16
    x = np.asarray(x, dtype=np.float32).astype(bf16)
    W_qkv = np.asarray(W_qkv, dtype=np.float32).astype(bf16)
    W_out = np.asarray(W_out, dtype=np.float32).astype(bf16)
    b_out = np.asarray(b_out, dtype=np.float32)

    nc = _get_program()
    in_maps = []
    for c in range(8):
        b, half = divmod(c, 2)
        xb = x[b]
        xrot = np.ascontiguousarray(
            np.concatenate([xb[half * NQ:], xb[:half * NQ]], axis=0))
        in_maps.append({"x": xrot, "wqkv": W_qkv, "wout": W_out,
                       "bout": b_out})
    res = run_bass_kernel_spmd(nc, in_maps, list(range(8)))
    out = np.empty((B, N, E), dtype=np.float32)
    for c in range(8):
        b, half = divmod(c, 2)
        out[b, half * NQ:(half + 1) * NQ] = res.results[c]["y"]
    return out


# revision 6
# speedup vs baseline: 1.2986x; 1.0670x over previous
"""Classical self-attention (head-summed scores) on 8 trn2 NeuronCores.

Math (per batch b):
    Q = x Wq; K = x Wk; V = x Wv          (W_qkv split columns 3x1024)
    S = Q K^T / 8   (full-E contraction: heads+dims summed)
    P = softmax(S, axis=-1)
    out = (P V) W_out + b_out

Sharding: 8 cores = (4 batches) x (2 query-halves). Each core gets its
batch's x rotated so its 1024 query rows come first; keys are the full
2048 rows (key order is irrelevant to the result). K/V projections are
duplicated between the 2 cores of a batch; no collectives needed.

v2 design (vs v1):
  - Inputs host-converted to bf16: halves DMA and SBUF, same PE rate
    (1.0 cycles/row) as fp32r in the TRN2 cost model.
  - x^T produced by DMA-transpose (XBAR) straight from DRAM: no PE
    transposes, no x staging in SBUF. Same-type DMAs are grouped (the
    tile framework serializes DMA streams at transpose<->copy mode
    switches), with Wk loads first so the warmup K matmuls can start
    as soon as the first x^T tile lands.
  - K^T, V, Q^T, P all SBUF-resident in bf16 -- no DRAM staging.
  - All matmuls emit <=512-element moving patterns (ISA limit).
  - Warmup: first 4 K^T chunks run f-outer so PE streams while x^T
    tiles are still arriving from DRAM.
  - Softmax skips max-subtraction (scores ~ N(0,16) after 1/8 scale);
    1/rowsum is deferred to the output projection.
  - ACT does only the exps (and half the O^T copies); DVE does the
    PSUM->SBUF copies; Pool does the bias adds. Output tail is split
    into 512-wide halves to shorten the post-PE drain.
"""

import sys

sys.path.insert(0, "/opt/trn_rl_repo")

import numpy as np

import concourse.bass as bass
import concourse.mybir as mybir
import concourse.tile as tile
from concourse import bacc

B, N, E = 4, 2048, 1024
NQ = N // 2          # query rows per core
P = 128              # partitions
FT = E // P          # 8 feature (contraction) tiles
ET = E // P          # 8 embed tiles
MT = N // P          # 16 key tiles
QT = NQ // P         # 8 query tiles
HKEY = N // 2        # 1024 keys per half (x^T transpose granularity)
MMF = 512            # max moving elements per matmul instruction
F32 = mybir.dt.float32
BF16 = mybir.dt.bfloat16
EXP = mybir.ActivationFunctionType.Exp


def build_program():
    nc = bacc.Bacc("TRN2", target_bir_lowering=False, debug=False)
    x = nc.dram_tensor("x", [N, E], BF16, kind="ExternalInput").ap()
    wqkv = nc.dram_tensor("wqkv", [E, 3 * E], BF16, kind="ExternalInput").ap()
    wout = nc.dram_tensor("wout", [E, E], BF16, kind="ExternalInput").ap()
    bout = nc.dram_tensor("bout", [E], F32, kind="ExternalInput").ap()
    y = nc.dram_tensor("y", [NQ, E], F32, kind="ExternalOutput").ap()

    with tile.TileContext(nc) as tc:
        _body(nc, tc, x, wqkv, wout, bout, y)
    nc.compile()
    return nc


def _mm(nc, out, lhsT, rhs, start, stop, width):
    """Accumulating matmul split into <=512-wide moving chunks."""
    for c0 in range(0, width, MMF):
        nc.tensor.matmul(out[:, c0:c0 + MMF], lhsT, rhs[:, c0:c0 + MMF],
                         start=start, stop=stop)


def _body(nc, tc, x, wqkv, wout, bout, y):
    # ---- persistent SBUF residents (right side) --------------------------
    wop = tc.alloc_tile_pool(name="wo", bufs=1, side="right")
    wo = [wop.tile([P, E], BF16, name=f"wo{e}", tag=f"wo{e}") for e in range(ET)]
    bo_b = wop.tile([P, E], F32, name="bo_b", tag="bo_b")

    kqp = tc.alloc_tile_pool(name="kq", bufs=1, side="right")
    kT = [kqp.tile([P, N], BF16, name=f"kT{e}", tag=f"kT{e}") for e in range(ET)]
    qT = [kqp.tile([P, NQ], BF16, name=f"qT{e}", tag=f"qT{e}") for e in range(ET)]

    vp = tc.alloc_tile_pool(name="vp", bufs=1, side="right")
    v = [vp.tile([P, E], BF16, name=f"v{m}", tag=f"v{m}") for m in range(MT)]

    smp = tc.alloc_tile_pool(name="small", bufs=1, side="right")
    ones = smp.tile([P, 1], BF16, name="ones", tag="ones")
    sums_acc = smp.tile([P, QT], F32, name="sums_acc", tag="sums_acc")
    recip = smp.tile([P, QT], F32, name="recip", tag="recip")

    # ---- phase 1: load + project ----------------------------------------
    wp = tc.alloc_tile_pool(name="wqkv", bufs=1)
    wk = [wp.tile([P, E], BF16, name=f"wk{f}", tag=f"wk{f}") for f in range(FT)]
    wq = [wp.tile([P, E], BF16, name=f"wq{f}", tag=f"wq{f}") for f in range(FT)]
    wv = [wp.tile([P, E], BF16, name=f"wv{f}", tag=f"wv{f}") for f in range(FT)]

    xTp = tc.alloc_tile_pool(name="xT", bufs=1)
    # xT[f][h]: [128, 1024] = transpose of x[h*1024:(h+1)*1024, f*128:(f+1)*128]
    xT = [[xTp.tile([P, HKEY], BF16, name=f"xT{f}_{h}", tag=f"xT{f}_{h}")
           for h in range(2)] for f in range(FT)]

    # DMA order (one queue; transfers execute in this order). Wk first so
    # the warmup matmuls are gated only by the x^T transposes, which land
    # one per ~0.9us -- slower than the 1.7us/round the warmup consumes.
    # Same-type DMAs stay contiguous: a transpose<->copy switch serializes
    # the two transfers back-to-back (XBAR mode drain).
    for f in range(FT):
        nc.sync.dma_start(out=wk[f], in_=wqkv[f * P:(f + 1) * P, E:2 * E])
    for h in range(2):
        for f in range(FT):
            nc.sync.dma_start_transpose(
                out=xT[f][h], in_=x[h * HKEY:(h + 1) * HKEY, f * P:(f + 1) * P])
    for f in range(FT):
        nc.sync.dma_start(out=wq[f], in_=wqkv[f * P:(f + 1) * P, 0:E])
    for f in range(FT):
        nc.sync.dma_start(out=wv[f], in_=wqkv[f * P:(f + 1) * P, 2 * E:3 * E])
    for e in range(ET):
        nc.sync.dma_start(out=wo[e], in_=wout[e * P:(e + 1) * P, :])
    bout_bcast = bass.AP(tensor=bout.tensor, offset=0, ap=[[0, P], [1, E]])
    nc.sync.dma_start(out=bo_b, in_=bout_bcast)
    nc.vector.memset(ones, 1.0)

    with tc.tile_pool(name="pjps", bufs=4, space="PSUM") as pjp:
        # Warmup: first 4 K chunks (e=0..3, keys half 0) f-outer.
        warm = [pjp.tile([P, HKEY], F32, name=f"pw{e}", tag="pj")
                for e in range(4)]
        for f in range(FT):
            for e in range(4):
                _mm(nc, warm[e], wk[f][:, e * P:(e + 1) * P], xT[f][0],
                    start=(f == 0), stop=(f == FT - 1), width=HKEY)
        for e in range(4):
            nc.vector.tensor_copy(kT[e][:, 0:HKEY], warm[e])

        # Remaining K chunks (f-inner), then Q, then V.
        for (e, h) in [(e, h) for h in range(2) for e in range(ET)
                       if not (h == 0 and e < 4)]:
            ps = pjp.tile([P, HKEY], F32, name="pjk", tag="pj")
            for f in range(FT):
                _mm(nc, ps, wk[f][:, e * P:(e + 1) * P], xT[f][h],
                    start=(f == 0), stop=(f == FT - 1), width=HKEY)
            nc.vector.tensor_copy(kT[e][:, h * HKEY:(h + 1) * HKEY], ps)

        for e in range(ET):
            ps = pjp.tile([P, NQ], F32, name="pjq", tag="pj")
            for f in range(FT):
                _mm(nc, ps, wq[f][:, e * P:(e + 1) * P], xT[f][0],
                    start=(f == 0), stop=(f == FT - 1), width=NQ)
            nc.vector.tensor_copy(qT[e], ps)

        for m in range(MT):
            h, mm_ = divmod(m, ET)
            ps = pjp.tile([P, E], F32, name="pjv", tag="pj")
            for f in range(FT):
                _mm(nc, ps, xT[f][h][:, mm_ * P:(mm_ + 1) * P], wv[f],
                    start=(f == 0), stop=(f == FT - 1), width=E)
            nc.vector.tensor_copy(v[m], ps)

    xTp.release()
    wp.release()

    # ---- phase 2: scores + exp + row-sums --------------------------------
    pp = tc.alloc_tile_pool(name="pp", bufs=1)
    p_tiles = [pp.tile([P, NQ], BF16, name=f"p{m}", tag=f"p{m}")
               for m in range(MT)]

    with tc.tile_pool(name="sps", bufs=3, space="PSUM") as sp, \
         tc.tile_pool(name="sumps", bufs=2, space="PSUM") as sumsp:
        for m in range(MT):
            s = sp.tile([P, NQ], F32, name="s", tag="s")
            for e in range(ET):
                _mm(nc, s, kT[e][:, m * P:(m + 1) * P], qT[e],
                    start=(e == 0), stop=(e == ET - 1), width=NQ)
            nc.scalar.activation(p_tiles[m], s, EXP, scale=0.125)
            # Row-sum the PREVIOUS tile's exp while this tile's S matmuls
            # occupy PE, so PE never waits on ACT.
            if m > 0:
                _row_sums(nc, p_tiles[m - 1], sumsp, ones, sums_acc,
                          first=(m == 1))
        _row_sums(nc, p_tiles[MT - 1], sumsp, ones, sums_acc, first=False)
        nc.vector.reciprocal(recip, sums_acc)

    # ---- phase 3: O^T = sum_m V[m]^T P^T[m] ------------------------------
    oTp = tc.alloc_tile_pool(name="oTp", bufs=1)
    oT = [oTp.tile([P, NQ], BF16, name=f"oT{e}", tag=f"oT{e}")
          for e in range(ET)]
    EG = ET // 2
    with tc.tile_pool(name="ops", bufs=1, space="PSUM") as opp:
        for g in range(2):
            o_ps = [opp.tile([P, NQ], F32, name=f"o{j}", tag=f"o{j}")
                    for j in range(EG)]
            for m in range(MT):
                for j in range(EG):
                    e = g * EG + j
                    _mm(nc, o_ps[j], v[m][:, e * P:(e + 1) * P], p_tiles[m],
                        start=(m == 0), stop=(m == MT - 1), width=NQ)
            # Alternate copy engines so group 1 gets PSUM banks back fast.
            for j in range(EG):
                e = g * EG + j
                if j % 2 == 0:
                    nc.vector.tensor_copy(oT[e], o_ps[j])
                else:
                    nc.scalar.copy(out=oT[e], in_=o_ps[j])

    # ---- phase 4: y = (O^T.T W_out) * recip + b_out ----------------------
    H2 = E // 2
    with tc.tile_pool(name="ysb", bufs=3) as ysp, \
         tc.tile_pool(name="yps", bufs=2, space="PSUM") as ypp:
        for nqt in range(QT):
            yps = ypp.tile([P, E], F32, name="yps", tag="yps")
            for e in range(ET):
                _mm(nc, yps, oT[e][:, nqt * P:(nqt + 1) * P], wo[e],
                    start=(e == 0), stop=(e == ET - 1), width=E)
            ysb = ysp.tile([P, E], F32, name="ysb", tag="ysb")
            # 512-wide halves: shortens the drain after the last matmul.
            for hh in range(2):
                sl = slice(hh * H2, (hh + 1) * H2)
                nc.vector.tensor_scalar_mul(ysb[:, sl], yps[:, sl],
                                            recip[:, nqt:nqt + 1])
                nc.gpsimd.tensor_add(out=ysb[:, sl], in0=ysb[:, sl],
                                     in1=bo_b[:, sl])
                nc.sync.dma_start(out=y[nqt * P:(nqt + 1) * P, sl],
                                  in_=ysb[:, sl])

    oTp.release()
    pp.release()
    smp.release()
    vp.release()
    kqp.release()
    wop.release()


def _row_sums(nc, p, sumsp, ones, sums_acc, first):
    sums_m = sumsp.tile([P, QT], F32, name="sums_m", tag="sums_m")
    for q in range(QT):
        nc.tensor.matmul(sums_m[:, q:q + 1], p[:, q * P:(q + 1) * P], ones,
                         start=True, stop=True)
    if first:
        nc.vector.tensor_copy(sums_acc, sums_m)
    else:
        nc.vector.tensor_tensor(out=sums_acc, in0=sums_acc,
                                in1=sums_m, op=mybir.AluOpType.add)


_NC_CACHE = None


def _get_program():
    global _NC_CACHE
    if _NC_CACHE is None:
        _NC_CACHE = build_program()
    return _NC_CACHE


def kernel(x, W_qkv, W_out, b_out):
    import ml_dtypes
    from concourse.bass_utils import run_bass_kernel_spmd

    bf16 = ml_dtypes.bfloat16
    x = np.asarray(x, dtype=np.float32).astype(bf16)
    W_qkv = np.asarray(W_qkv, dtype=np.float32).astype(bf16)
    W_out = np.asarray(W_out, dtype=np.float32).astype(bf16)
    b_out = np.asarray(b_out, dtype=np.float32)

    nc = _get_program()
    in_maps = []
    for c in range(8):
        b, half = divmod(c, 2)
        xb = x[b]
        xrot = np.ascontiguousarray(
            np.concatenate([xb[half * NQ:], xb[:half * NQ]], axis=0))
        in_maps.append({"x": xrot, "wqkv": W_qkv, "wout": W_out,
                       "bout": b_out})
    res = run_bass_kernel_spmd(nc, in_maps, list(range(8)))
    out = np.empty((B, N, E), dtype=np.float32)
    for c in range(8):
        b, half = divmod(c, 2)
        out[b, half * NQ:(half + 1) * NQ] = res.results[c]["y"]
    return out


# revision 11
# speedup vs baseline: 1.3055x; 1.0053x over previous
"""Classical self-attention (head-summed scores) on 8 trn2 NeuronCores.

Math (per batch b):
    Q = x Wq; K = x Wk; V = x Wv          (W_qkv split columns 3x1024)
    S = Q K^T / 8   (full-E contraction: heads+dims summed)
    P = softmax(S, axis=-1)
    out = (P V) W_out + b_out

Sharding: 8 cores = (4 batches) x (2 query-halves). Each core gets its
batch's x rotated so its 1024 query rows come first; keys are the full
2048 rows (key order is irrelevant to the result). K/V projections are
duplicated between the 2 cores of a batch; no collectives needed.

v2 design (vs v1):
  - Inputs host-converted to bf16: halves DMA and SBUF, same PE rate
    (1.0 cycles/row) as fp32r in the TRN2 cost model.
  - x^T produced by DMA-transpose (XBAR) straight from DRAM: no PE
    transposes, no x staging in SBUF. Same-type DMAs are grouped (the
    tile framework serializes DMA streams at transpose<->copy mode
    switches), with Wk loads first so the warmup K matmuls can start
    as soon as the first x^T tile lands.
  - K^T, V, Q^T, P all SBUF-resident in bf16 -- no DRAM staging.
  - All matmuls emit <=512-element moving patterns (ISA limit).
  - Warmup: first 4 K^T chunks run f-outer so PE streams while x^T
    tiles are still arriving from DRAM.
  - Softmax skips max-subtraction (scores ~ N(0,16) after 1/8 scale);
    1/rowsum is deferred to the output projection.
  - ACT does only the exps (and half the O^T copies); DVE does the
    PSUM->SBUF copies; Pool does the bias adds. Output tail is split
    into 512-wide halves to shorten the post-PE drain.
"""

import sys

sys.path.insert(0, "/opt/trn_rl_repo")

import numpy as np

import concourse.bass as bass
import concourse.mybir as mybir
import concourse.tile as tile
from concourse import bacc

B, N, E = 4, 2048, 1024
NQ = N // 2          # query rows per core
P = 128              # partitions
FT = E // P          # 8 feature (contraction) tiles
ET = E // P          # 8 embed tiles
MT = N // P          # 16 key tiles
QT = NQ // P         # 8 query tiles
HKEY = N // 2        # 1024 keys per half (x^T transpose granularity)
MMF = 512            # max moving elements per matmul instruction
F32 = mybir.dt.float32
BF16 = mybir.dt.bfloat16
EXP = mybir.ActivationFunctionType.Exp


def build_program():
    nc = bacc.Bacc("TRN2", target_bir_lowering=False, debug=False)
    x = nc.dram_tensor("x", [N, E], BF16, kind="ExternalInput").ap()
    wqkv = nc.dram_tensor("wqkv", [E, 3 * E], BF16, kind="ExternalInput").ap()
    wout = nc.dram_tensor("wout", [E, E], BF16, kind="ExternalInput").ap()
    bout = nc.dram_tensor("bout", [E], F32, kind="ExternalInput").ap()
    y = nc.dram_tensor("y", [NQ, E], F32, kind="ExternalOutput").ap()

    with tile.TileContext(nc) as tc:
        _body(nc, tc, x, wqkv, wout, bout, y)
    nc.compile()
    return nc


def _mm(nc, out, lhsT, rhs, start, stop, width):
    """Accumulating matmul split into <=512-wide moving chunks."""
    for c0 in range(0, width, MMF):
        nc.tensor.matmul(out[:, c0:c0 + MMF], lhsT, rhs[:, c0:c0 + MMF],
                         start=start, stop=stop)


def _body(nc, tc, x, wqkv, wout, bout, y):
    # ---- persistent SBUF residents (right side) --------------------------
    wop = tc.alloc_tile_pool(name="wo", bufs=1, side="right")
    wo = [wop.tile([P, E], BF16, name=f"wo{e}", tag=f"wo{e}") for e in range(ET)]
    bo_b = wop.tile([P, E], F32, name="bo_b", tag="bo_b")

    kqp = tc.alloc_tile_pool(name="kq", bufs=1, side="right")
    kT = [kqp.tile([P, N], BF16, name=f"kT{e}", tag=f"kT{e}") for e in range(ET)]
    qT = [kqp.tile([P, NQ], BF16, name=f"qT{e}", tag=f"qT{e}") for e in range(ET)]

    vp = tc.alloc_tile_pool(name="vp", bufs=1, side="right")
    v = [vp.tile([P, E], BF16, name=f"v{m}", tag=f"v{m}") for m in range(MT)]

    smp = tc.alloc_tile_pool(name="small", bufs=1, side="right")
    ones = smp.tile([P, 1], BF16, name="ones", tag="ones")
    sums_acc = smp.tile([P, QT], F32, name="sums_acc", tag="sums_acc")
    recip = smp.tile([P, QT], F32, name="recip", tag="recip")

    # ---- phase 1: load + project ----------------------------------------
    wp = tc.alloc_tile_pool(name="wqkv", bufs=1)
    wk = [wp.tile([P, E], BF16, name=f"wk{f}", tag=f"wk{f}") for f in range(FT)]
    wq = [wp.tile([P, E], BF16, name=f"wq{f}", tag=f"wq{f}") for f in range(FT)]
    wv = [wp.tile([P, E], BF16, name=f"wv{f}", tag=f"wv{f}") for f in range(FT)]

    xTp = tc.alloc_tile_pool(name="xT", bufs=1)
    # xT[f][h]: [128, 1024] = transpose of x[h*1024:(h+1)*1024, f*128:(f+1)*128]
    xT = [[xTp.tile([P, HKEY], BF16, name=f"xT{f}_{h}", tag=f"xT{f}_{h}")
           for h in range(2)] for f in range(FT)]

    # DMA order (one queue; transfers execute in this order). Wk0-3 first
    # so the warmup matmuls are gated only by the x^T transposes, which
    # land one per ~0.9us -- slower than the 1.7us/round the warmup
    # consumes; wk4-7 slot in after half 0 (0.73us/tile, still ahead of
    # the rounds that need them). Same-type DMAs stay contiguous: a
    # transpose<->copy switch serializes the two transfers back-to-back
    # (XBAR mode drain).
    for f in range(4):
        nc.sync.dma_start(out=wk[f], in_=wqkv[f * P:(f + 1) * P, E:2 * E])
    for f in range(FT):
        nc.sync.dma_start_transpose(
            out=xT[f][0], in_=x[0:HKEY, f * P:(f + 1) * P])
    for f in range(4, FT):
        nc.sync.dma_start(out=wk[f], in_=wqkv[f * P:(f + 1) * P, E:2 * E])
    for f in range(FT):
        nc.sync.dma_start_transpose(
            out=xT[f][1], in_=x[HKEY:N, f * P:(f + 1) * P])
    for f in range(FT):
        nc.sync.dma_start(out=wq[f], in_=wqkv[f * P:(f + 1) * P, 0:E])
    for f in range(FT):
        nc.sync.dma_start(out=wv[f], in_=wqkv[f * P:(f + 1) * P, 2 * E:3 * E])
    for e in range(ET):
        nc.sync.dma_start(out=wo[e], in_=wout[e * P:(e + 1) * P, :])
    bout_bcast = bass.AP(tensor=bout.tensor, offset=0, ap=[[0, P], [1, E]])
    nc.sync.dma_start(out=bo_b, in_=bout_bcast)
    nc.vector.memset(ones, 1.0)

    with tc.tile_pool(name="pjps", bufs=4, space="PSUM") as pjp:
        # Warmup: first 4 K chunks (e=0..3, keys half 0) f-outer.
        warm = [pjp.tile([P, HKEY], F32, name=f"pw{e}", tag="pj")
                for e in range(4)]
        for f in range(FT):
            for e in range(4):
                _mm(nc, warm[e], wk[f][:, e * P:(e + 1) * P], xT[f][0],
                    start=(f == 0), stop=(f == FT - 1), width=HKEY)
        for e in range(4):
            nc.vector.tensor_copy(kT[e][:, 0:HKEY], warm[e])

        # Remaining K chunks (f-inner), then Q, then V.
        for (e, h) in [(e, h) for h in range(2) for e in range(ET)
                       if not (h == 0 and e < 4)]:
            ps = pjp.tile([P, HKEY], F32, name="pjk", tag="pj")
            for f in range(FT):
                _mm(nc, ps, wk[f][:, e * P:(e + 1) * P], xT[f][h],
                    start=(f == 0), stop=(f == FT - 1), width=HKEY)
            nc.vector.tensor_copy(kT[e][:, h * HKEY:(h + 1) * HKEY], ps)

        for e in range(ET):
            ps = pjp.tile([P, NQ], F32, name="pjq", tag="pj")
            for f in range(FT):
                _mm(nc, ps, wq[f][:, e * P:(e + 1) * P], xT[f][0],
                    start=(f == 0), stop=(f == FT - 1), width=NQ)
            nc.vector.tensor_copy(qT[e], ps)

        for m in range(MT):
            h, mm_ = divmod(m, ET)
            ps = pjp.tile([P, E], F32, name="pjv", tag="pj")
            for f in range(FT):
                _mm(nc, ps, xT[f][h][:, mm_ * P:(mm_ + 1) * P], wv[f],
                    start=(f == 0), stop=(f == FT - 1), width=E)
            if m < MT - 1:
                nc.vector.tensor_copy(v[m], ps)
            else:
                # Last projection drain gates the scores PSUM pool: split
                # across DVE+ACT so it clears in half the time.
                nc.vector.tensor_copy(v[m][:, 0:E // 2], ps[:, 0:E // 2])
                nc.scalar.copy(out=v[m][:, E // 2:E], in_=ps[:, E // 2:E])

    xTp.release()
    wp.release()

    # ---- phase 2: scores + exp + row-sums --------------------------------
    pp = tc.alloc_tile_pool(name="pp", bufs=1)
    p_tiles = [pp.tile([P, NQ], BF16, name=f"p{m}", tag=f"p{m}")
               for m in range(MT)]

    with tc.tile_pool(name="sps", bufs=3, space="PSUM") as sp, \
         tc.tile_pool(name="sumps", bufs=2, space="PSUM") as sumsp:
        for m in range(MT):
            s = sp.tile([P, NQ], F32, name="s", tag="s")
            for e in range(ET):
                _mm(nc, s, kT[e][:, m * P:(m + 1) * P], qT[e],
                    start=(e == 0), stop=(e == ET - 1), width=NQ)
            if m < MT - 1:
                nc.scalar.activation(p_tiles[m], s, EXP, scale=0.125)
            else:
                # Last exp gates the PV PSUM pool: split into halves.
                for hh in range(2):
                    sl = slice(hh * (NQ // 2), (hh + 1) * (NQ // 2))
                    nc.scalar.activation(p_tiles[m][:, sl], s[:, sl], EXP,
                                         scale=0.125)
            # Row-sum the PREVIOUS tile's exp while this tile's S matmuls
            # occupy PE, so PE never waits on ACT.
            if m > 0:
                _row_sums(nc, p_tiles[m - 1], sumsp, ones, sums_acc,
                          first=(m == 1))
        _row_sums(nc, p_tiles[MT - 1], sumsp, ones, sums_acc, first=False)
        nc.vector.reciprocal(recip, sums_acc)

    # ---- phase 3: O^T = sum_m V[m]^T P^T[m] ------------------------------
    oTp = tc.alloc_tile_pool(name="oTp", bufs=1)
    oT = [oTp.tile([P, NQ], BF16, name=f"oT{e}", tag=f"oT{e}")
          for e in range(ET)]
    EG = ET // 2
    with tc.tile_pool(name="ops", bufs=1, space="PSUM") as opp:
        for g in range(2):
            o_ps = [opp.tile([P, NQ], F32, name=f"o{j}", tag=f"o{j}")
                    for j in range(EG)]
            for m in range(MT):
                for j in range(EG):
                    e = g * EG + j
                    _mm(nc, o_ps[j], v[m][:, e * P:(e + 1) * P], p_tiles[m],
                        start=(m == 0), stop=(m == MT - 1), width=NQ)
            # Alternate copy engines (and split halves on the last group)
            # so the next phase gets PSUM banks back fast.
            for j in range(EG):
                e = g * EG + j
                if g == 0:
                    if j % 2 == 0:
                        nc.vector.tensor_copy(oT[e], o_ps[j])
                    else:
                        nc.scalar.copy(out=oT[e], in_=o_ps[j])
                else:
                    h2 = NQ // 2
                    nc.vector.tensor_copy(oT[e][:, 0:h2], o_ps[j][:, 0:h2])
                    nc.scalar.copy(out=oT[e][:, h2:NQ], in_=o_ps[j][:, h2:NQ])

    # ---- phase 4: y = (O^T.T W_out) * recip + b_out ----------------------
    H2 = E // 2
    with tc.tile_pool(name="ysb", bufs=3) as ysp, \
         tc.tile_pool(name="yps", bufs=2, space="PSUM") as ypp:
        for nqt in range(QT):
            yps = ypp.tile([P, E], F32, name="yps", tag="yps")
            for e in range(ET):
                _mm(nc, yps, oT[e][:, nqt * P:(nqt + 1) * P], wo[e],
                    start=(e == 0), stop=(e == ET - 1), width=E)
            ysb = ysp.tile([P, E], F32, name="ysb", tag="ysb")
            # Fused (yps * recip) + b_out on DVE, in 512-wide halves:
            # shortens the drain after the last matmul.
            for hh in range(2):
                sl = slice(hh * H2, (hh + 1) * H2)
                nc.vector.scalar_tensor_tensor(
                    out=ysb[:, sl], in0=yps[:, sl],
                    scalar=recip[:, nqt:nqt + 1], in1=bo_b[:, sl],
                    op0=mybir.AluOpType.mult, op1=mybir.AluOpType.add)
                nc.sync.dma_start(out=y[nqt * P:(nqt + 1) * P, sl],
                                  in_=ysb[:, sl])

    oTp.release()
    pp.release()
    smp.release()
    vp.release()
    kqp.release()
    wop.release()


def _row_sums(nc, p, sumsp, ones, sums_acc, first):
    sums_m = sumsp.tile([P, QT], F32, name="sums_m", tag="sums_m")
    for q in range(QT):
        nc.tensor.matmul(sums_m[:, q:q + 1], p[:, q * P:(q + 1) * P], ones,
                         start=True, stop=True)
    if first:
        nc.vector.tensor_copy(sums_acc, sums_m)
    else:
        nc.vector.tensor_tensor(out=sums_acc, in0=sums_acc,
                                in1=sums_m, op=mybir.AluOpType.add)


_NC_CACHE = None


def _get_program():
    global _NC_CACHE
    if _NC_CACHE is None:
        _NC_CACHE = build_program()
    return _NC_CACHE


def kernel(x, W_qkv, W_out, b_out):
    import ml_dtypes
    from concourse.bass_utils import run_bass_kernel_spmd

    bf16 = ml_dtypes.bfloat16
    x = np.asarray(x, dtype=np.float32).astype(bf16)
    W_qkv = np.asarray(W_qkv, dtype=np.float32).astype(bf16)
    W_out = np.asarray(W_out, dtype=np.float32).astype(bf16)
    b_out = np.asarray(b_out, dtype=np.float32)

    nc = _get_program()
    in_maps = []
    for c in range(8):
        b, half = divmod(c, 2)
        xb = x[b]
        xrot = np.ascontiguousarray(
            np.concatenate([xb[half * NQ:], xb[:half * NQ]], axis=0))
        in_maps.append({"x": xrot, "wqkv": W_qkv, "wout": W_out,
                       "bout": b_out})
    res = run_bass_kernel_spmd(nc, in_maps, list(range(8)))
    out = np.empty((B, N, E), dtype=np.float32)
    for c in range(8):
        b, half = divmod(c, 2)
        out[b, half * NQ:(half + 1) * NQ] = res.results[c]["y"]
    return out
